# revision 27
# baseline (speedup 1.0000x reference)
"""BiDAF attention layer on 8 Trainium2 NeuronCores (Bass/Tile).

Math (per batch b):
  t[i,j]  = sum_d (c[i,d]*w_cq[d] + w_q[d]) * q[j,d]   (= cq + sq0[j])
  a       = softmax_j(t)            (biases b_c/b_q/b_cq cancel in softmax)
  c2q     = a @ q
  m[i]    = max_j t[i,j];  sc0[i] = c[i,:]@w_c
  bvec    = softmax_i(m + sc0)      (biases cancel here too)
  q2c     = bvec @ c
  out     = [c | c2q | c*c2q | c*q2c]

Sharding: data-parallel over batch, 4 batches per core, params replicated.

Structure:
  - scores computed once, in [j,i] layout only.  exp() emitted per jc-pair
    so ACT overlaps the PE score matmuls.
  - row maxes via max(e) = e^max: pairwise fp16 DVE max over the jc blocks
    of eT -> emax[j',i]; per-i-tile PE transpose + one reduce_max give
    emax_col = e^m.  bvec numerators are then emax_col * exp(sc0-2.5)
    (multiplicative sc0 fold; sc0 columns from 16 tiny N=1 matmuls).
  - w_cq/w_q folded into chatT on the PSUM evacuation of the c transposes
    (tensor_scalar), so the score contraction emits the q@w_q row term.
  - c*c2q fused as (po*linv)*c via scalar_tensor_tensor straight from the
    c2q PSUM; 2 of 4 tiles per half on gpsimd from the normalized stage
    (gpsimd cannot read PSUM).  c*q2c in fp16, cast f16->f32 in the
    output DMA (SWDGE).
  - software pipelining: the next batch's q/c transposes + sc0 matmuls are
    emitted in 7 chunks at fixed points inside the current batch's body so
    the PE never drains (and the HAM clock stays unthrottled -- PE
    transposes do not count as busy for the clock gate).
  - partition-major DRAM layout ((p t) d) so each partition's DMA is one
    contiguous 8 KB descriptor instead of 8 strided 1 KB ones.
  - PSUM: 4 banks for the half-scores accumulator, 4 banks as a single
    rotating bufs=4 pool for everything else (transposes, sc0, c2q po
    tiles, q2c) -- per-tag slot allocation makes one shared tag mandatory.
  - DMA split: c-in/c-out on sync(SP), stage-out on scalar(ACT), q-in and
    c_h-in (f32->fp16 cast) and c4-out on gpsimd(SWDGE).

Measured: 118.0 us HW exec (vs 136.4 us v1 baseline), rel err 1.1e-04.
"""

import sys

if "/opt/trn_rl_repo" not in sys.path:
    sys.path.insert(0, "/opt/trn_rl_repo")

import numpy as np

import concourse.bass as bass
import concourse.tile as tile
from concourse import bacc, mybir
from concourse.bass import ds, ts
from concourse.masks import make_identity

B, CL, QL, D = 32, 1024, 512, 256
NCORES = 8
BS = B // NCORES  # batches per core
P = 128
F32 = mybir.dt.float32
F16 = mybir.dt.float16

NT = CL // P  # 8 i-tiles
NJ = QL // P  # 4 j-chunks
ND = D // P   # 2 d-chunks
NH = 2        # i-halves for the [j,i]-layout score matmul
IH = CL // NH  # 512
KPH = NT // NH  # i-tiles per half

Exp = mybir.ActivationFunctionType.Exp
AxX = mybir.AxisListType.X
Mult = mybir.AluOpType.mult
Add = mybir.AluOpType.add
Max = mybir.AluOpType.max


def build_bass(bs: int = BS):
    nc = bacc.Bacc(None)
    c_d = nc.declare_dram_parameter("c", [bs, CL, D], F32, isOutput=False)
    q_d = nc.declare_dram_parameter("q", [bs, QL, D], F32, isOutput=False)
    wc_d = nc.declare_dram_parameter("wc_cols", [P, ND], F32, isOutput=False)
    wq_d = nc.declare_dram_parameter("wq_cols", [P, ND], F32, isOutput=False)
    wcq_d = nc.declare_dram_parameter("wcq_cols", [P, ND], F32, isOutput=False)
    out_d = nc.declare_dram_parameter("out", [bs, CL, 4 * D], F32, isOutput=True)

    with tile.TileContext(nc) as tc:
        with (
            tc.tile_pool(name="consts", bufs=1) as consts,
            tc.tile_pool(name="io", bufs=2) as io,
            tc.tile_pool(name="ins", bufs=4) as ins,
            tc.tile_pool(name="work", bufs=2) as work,
            tc.tile_pool(name="ps_tT", bufs=1, space="PSUM") as ps_tT,
            tc.tile_pool(name="ps_w", bufs=4, space="PSUM") as ps_w,
        ):
            ident_h = consts.tile([P, P], F16)
            ones_f = consts.tile([P, P], F32)
            ones_h = consts.tile([P, P], F16)
            wc_sb = consts.tile([P, ND], F32)
            wq_sb = consts.tile([P, ND], F32)
            wcq_sb = consts.tile([P, ND], F32)
            wc_h = consts.tile([P, ND], F16)
            neg_shift = consts.tile([P, 1], F32)

            def emit_inputs(b):
                # c loaded once in f32; block0 of out is c verbatim
                c_sb = ins.tile([P, NT, D], F32, tag="c_sb")
                nc.sync.dma_start(
                    out=c_sb, in_=c_d[b].rearrange("(p t) d -> p t d", p=P)
                )
                ov = out_d[b].rearrange("(p t) x -> p t x", p=P)
                q_sb = ins.tile([P, NJ, D + 1], F16, tag="q_sb")
                c_h = ins.tile([P, NT, D], F16, tag="c_h")
                # q/c_h cast f32 -> fp16 in-flight (SWDGE)
                nc.gpsimd.dma_start(
                    out=q_sb[:, :, 0:D],
                    in_=q_d[b].rearrange("(p t) d -> p t d", p=P),
                )
                nc.gpsimd.dma_start(
                    out=c_h, in_=c_d[b].rearrange("(p t) d -> p t d", p=P)
                )
                nc.vector.memset(q_sb[:, :, D : D + 1], 1.0)
                nc.sync.dma_start(out=ov[:, :, 0:D], in_=c_sb)
                return c_sb, q_sb, c_h, ov

            def head_alloc():
                qT = work.tile([P, ND, QL], F16, tag="qT")
                cT = work.tile([P, ND, CL], F16, tag="cT")
                chatT = work.tile([P, ND, CL], F16, tag="chatT")
                return {"qT": qT, "cT": cT, "chatT": chatT}

            def head_qT(st, q_sb, dc):
                qps = ps_w.tile([P, QL], F16, tag="w")
                for jc in range(NJ):
                    nc.tensor.transpose(
                        qps[:, ts(jc, P)], q_sb[:, jc, ts(dc, P)], ident_h
                    )
                if dc == 0:
                    nc.scalar.copy(st["qT"][:, dc], qps)
                else:
                    nc.vector.tensor_copy(st["qT"][:, dc], qps)

            def head_cT(st, c_h, dc, h):
                pst = ps_w.tile([P, IH], F16, tag="w")
                for k in range(KPH):
                    it = h * KPH + k
                    nc.tensor.transpose(
                        pst[:, ts(k, P)], c_h[:, it, ts(dc, P)], ident_h
                    )
                sl = ds(h * IH, IH)
                nc.vector.tensor_scalar(
                    out=st["chatT"][:, dc, sl],
                    in0=pst,
                    scalar1=wcq_sb[:, dc : dc + 1],
                    scalar2=wq_sb[:, dc : dc + 1],
                    op0=Mult,
                    op1=Add,
                )
                nc.scalar.copy(st["cT"][:, dc, sl], pst)

            def head_sc0(st):
                sc0ps = ps_w.tile([P, NT], F32, tag="w")
                for it in range(NT):
                    for dc in range(ND):
                        nc.tensor.matmul(
                            sc0ps[:, it : it + 1],
                            st["cT"][:, dc, ts(it, P)],
                            wc_h[:, dc : dc + 1],
                            start=(dc == 0),
                            stop=(dc == ND - 1),
                        )
                esc = work.tile([P, NT], F32, tag="esc")
                nc.scalar.activation(esc, sc0ps, Exp, bias=neg_shift[:, 0:1])
                st["esc"] = esc

            def emit_head_full(q_sb, c_h):
                st = head_alloc()
                for dc in range(ND):
                    head_qT(st, q_sb, dc)
                for dc in range(ND):
                    for h in range(NH):
                        head_cT(st, c_h, dc, h)
                head_sc0(st)
                return st

            nc.scalar.dma_start(out=wc_sb, in_=wc_d[:])
            nc.scalar.dma_start(out=wq_sb, in_=wq_d[:])
            nc.scalar.dma_start(out=wcq_sb, in_=wcq_d[:])
            make_identity(nc, ident_h)
            nc.vector.memset(ones_f, 1.0)
            nc.vector.memset(ones_h, 1.0)
            nc.vector.memset(neg_shift, -2.5)
            nc.vector.tensor_copy(wc_h, wc_sb)

            pending = [emit_inputs(0)]

            for b in range(bs):
                c_sb, q_sb, c_h, ov = pending.pop(0)

                # prefetch up to two batches ahead
                if b == 0:
                    for nb in (1, 2, 3):
                        if nb < bs:
                            pending.append(emit_inputs(nb))
                elif b + 3 < bs:
                    pending.append(emit_inputs(b + 3))

                if b == 0:
                    cur = emit_head_full(q_sb, c_h)
                if b + 1 < bs:
                    nxt = head_alloc()
                    nq, nch = pending[0][1], pending[0][2]
                    chunks = [
                        lambda: head_qT(nxt, nq, 0),
                        lambda: head_qT(nxt, nq, 1),
                        lambda: head_cT(nxt, nch, 0, 0),
                        lambda: head_cT(nxt, nch, 0, 1),
                        lambda: head_cT(nxt, nch, 1, 0),
                        lambda: head_cT(nxt, nch, 1, 1),
                        lambda: head_sc0(nxt),
                    ]
                else:
                    nxt, chunks = None, []

                def pump():
                    if chunks:
                        chunks.pop(0)()

                qT, cT, chatT, esc = (
                    cur["qT"], cur["cT"], cur["chatT"], cur["esc"]
                )
                emax = work.tile([P, CL], F16, tag="emax")
                emax_col = work.tile([P, NT], F32, tag="emaxc")
                ebv_h = work.tile([P, NT], F16, tag="ebvh")
                stage = io.tile([P, NT, 2 * D], F32, tag="stage")
                eTs = []

                for h in range(NH):
                    isl = ds(h * KPH, KPH)
                    csl = ds(h * IH, IH)
                    # ---- scores tT[j, i] for this half, exp per jc ----
                    tTq = ps_tT.tile([P, NJ, IH], F32, tag="tTq")
                    eT = work.tile([P, NJ, IH], F16, tag=f"eT{h}")
                    eTs.append(eT)
                    for jc in range(NJ):
                        for dc in range(ND):
                            nc.tensor.matmul(
                                tTq[:, jc],
                                qT[:, dc, ts(jc, P)],
                                chatT[:, dc, csl],
                                start=(dc == 0),
                                stop=(dc == ND - 1),
                            )
                        if jc % 2 == 1:
                            nc.scalar.activation(
                                eT[:, jc - 1 : jc + 1],
                                tTq[:, jc - 1 : jc + 1],
                                Exp,
                            )
                    pump()
                    # row max of e over the 4 jc blocks (fp16 SBUF; max of
                    # exp == exp of max, so emax = e^m directly)
                    tm0 = work.tile([P, IH], F16, tag="tm0")
                    nc.vector.tensor_tensor(tm0, eT[:, 0], eT[:, 1], op=Max)
                    tm1 = work.tile([P, IH], F16, tag="tm1")
                    nc.vector.tensor_tensor(tm1, eT[:, 2], eT[:, 3], op=Max)
                    nc.vector.tensor_tensor(emax[:, csl], tm0, tm1, op=Max)

                    # ---- e^m columns for this half: per-tile transpose of
                    # emax, then one reduce_max over j' ----
                    mps = ps_w.tile([P, KPH, P], F16, tag="w")
                    for k in range(KPH):
                        it = h * KPH + k
                        nc.tensor.transpose(
                            mps[:, k], emax[:, ts(it, P)], ident_h
                        )
                    nc.vector.reduce_max(emax_col[:, isl], mps, AxX)
                    nc.vector.tensor_mul(
                        ebv_h[:, isl], emax_col[:, isl], esc[:, isl]
                    )
                    pump()

                    # ---- c2q for this half ----
                    for k in range(KPH):
                        it = h * KPH + k
                        po = ps_w.tile([P, D + 1], F32, tag="w")
                        for jc in range(NJ):
                            nc.tensor.matmul(
                                po,
                                eT[:, jc, ts(k, P)],
                                q_sb[:, jc],
                                start=(jc == 0),
                                stop=(jc == NJ - 1),
                            )
                        linv = work.tile([P, 1], F32, tag="linv")
                        nc.vector.reciprocal(linv, po[:, D : D + 1])
                        if k < 3:
                            nc.scalar.mul(stage[:, it, 0:D], po[:, 0:D], linv)
                        else:
                            nc.vector.tensor_scalar_mul(
                                stage[:, it, 0:D], po[:, 0:D], linv
                            )
                        if k < 2:
                            nc.vector.scalar_tensor_tensor(
                                out=stage[:, it, D : 2 * D],
                                in0=po[:, 0:D],
                                scalar=linv,
                                in1=c_sb[:, it],
                                op0=Mult,
                                op1=Mult,
                            )
                        else:
                            # gpsimd cannot read PSUM; use normalized stage1
                            nc.gpsimd.tensor_mul(
                                stage[:, it, D : 2 * D],
                                stage[:, it, 0:D],
                                c_sb[:, it],
                            )
                        if k == 1:
                            pump()
                    if h == 0:
                        nc.scalar.dma_start(
                            out=ov[:, isl, D : 3 * D], in_=stage[:, isl]
                        )
                        pump()
                    else:
                        half = KPH // 2
                        nc.scalar.dma_start(
                            out=ov[:, ds(h * KPH, half), D : 3 * D],
                            in_=stage[:, ds(h * KPH, half)],
                        )
                        nc.scalar.dma_start(
                            out=ov[:, ds(h * KPH + half, half), D : 3 * D],
                            in_=stage[:, ds(h * KPH + half, half)],
                        )

                while chunks:
                    pump()
                cur = nxt

                # ---- q2c chain ----
                colsum = work.tile([P, 1], F32, tag="colsum")
                nc.vector.reduce_sum(colsum, ebv_h, AxX)
                ps_tot = ps_w.tile([P, 1], F32, tag="w")
                nc.tensor.matmul(ps_tot, ones_f, colsum, start=True, stop=True)
                totinv = work.tile([P, 1], F32, tag="totinv")
                nc.vector.reciprocal(totinv, ps_tot)
                ps_q2c = ps_w.tile([1, D], F32, tag="w")
                for it in range(NT):
                    nc.tensor.matmul(
                        ps_q2c,
                        ebv_h[:, it : it + 1],
                        c_h[:, it],
                        start=(it == 0),
                        stop=(it == NT - 1),
                    )
                q2c_row = work.tile([1, D], F32, tag="q2cr")
                nc.vector.tensor_scalar_mul(q2c_row, ps_q2c, totinv[0:1, 0:1])
                ps_q2cb = ps_w.tile([P, D], F32, tag="w")
                nc.tensor.matmul(
                    ps_q2cb, ones_f[0:1, :], q2c_row, start=True, stop=True
                )
                q2c_sb = work.tile([P, D], F16, tag="q2csb")
                nc.scalar.copy(q2c_sb, ps_q2cb)
                c4st = io.tile([P, NT, D], F16, tag="c4st")
                for it in range(KPH):
                    nc.vector.tensor_mul(c4st[:, it], c_h[:, it], q2c_sb)
                nc.gpsimd.dma_start(
                    out=ov[:, 0:KPH, 3 * D : 4 * D], in_=c4st[:, 0:KPH]
                )
                for it in range(KPH, NT):
                    nc.gpsimd.tensor_mul(c4st[:, it], c_h[:, it], q2c_sb)
                nc.gpsimd.dma_start(
                    out=ov[:, KPH:NT, 3 * D : 4 * D], in_=c4st[:, KPH:NT]
                )

    nc.compile()
    return nc


_NC_CACHE = {}


def _get_nc(bs: int = BS):
    if bs not in _NC_CACHE:
        _NC_CACHE[bs] = build_bass(bs)
    return _NC_CACHE[bs]


def _param_maps(w_c, w_q, w_cq):
    wc_cols = np.ascontiguousarray(np.asarray(w_c, np.float32).reshape(ND, P).T)
    wq_cols = np.ascontiguousarray(np.asarray(w_q, np.float32).reshape(ND, P).T)
    wcq_cols = np.ascontiguousarray(
        np.asarray(w_cq, np.float32).reshape(ND, P).T
    )
    return wc_cols, wq_cols, wcq_cols


def _run(c, q, w_c, w_q, w_cq, trace=False, **trace_kwargs):
    from concourse.bass_utils import run_bass_kernel_spmd

    c = np.asarray(c, np.float32)
    q = np.asarray(q, np.float32)
    wc_cols, wq_cols, wcq_cols = _param_maps(w_c, w_q, w_cq)

    nc = _get_nc(BS)
    in_maps = []
    for k in range(NCORES):
        in_maps.append(
            {
                "c": np.ascontiguousarray(c[k * BS : (k + 1) * BS]),
                "q": np.ascontiguousarray(q[k * BS : (k + 1) * BS]),
                "wc_cols": wc_cols,
                "wq_cols": wq_cols,
                "wcq_cols": wcq_cols,
            }
        )
    res = None
    last_err = None
    for attempt in range(3):
        try:
            res = run_bass_kernel_spmd(
                nc,
                in_maps,
                core_ids=list(range(NCORES)),
                trace=trace,
                **trace_kwargs,
            )
            break
        except Exception as e:  # transient device wedges clear on retry
            last_err = e
            if "UNRECOVERABLE" not in str(e) and "UNAVAILABLE" not in str(e):
                raise
    if res is None:
        raise last_err
    out = np.concatenate([res.results[k]["out"] for k in range(NCORES)], axis=0)
    return out, res


def kernel(c, q, w_c, b_c, w_q, b_q, w_cq, b_cq):
    # b_c/b_q/b_cq provably cancel in both softmaxes; output doesn't use them.
    out, _ = _run(c, q, w_c, w_q, w_cq)
    return out


# revision 28
# speedup vs baseline: 1.0140x; 1.0140x over previous
"""BiDAF attention layer on 8 Trainium2 NeuronCores (Bass/Tile).

Math (per batch b):
  t[i,j]  = sum_d (c[i,d]*w_cq[d] + w_q[d]) * q[j,d]   (= cq + sq0[j])
  a       = softmax_j(t)            (biases b_c/b_q/b_cq cancel in softmax)
  c2q     = a @ q
  m[i]    = max_j t[i,j];  sc0[i] = c[i,:]@w_c
  bvec    = softmax_i(m + sc0)      (biases cancel here too)
  q2c     = bvec @ c
  out     = [c | c2q | c*c2q | c*q2c]

Sharding: data-parallel over batch, 4 batches per core, params replicated.

Structure:
  - scores computed once, in [j,i] layout only.  exp() emitted per jc-pair
    so ACT overlaps the PE score matmuls.
  - row maxes via max(e) = e^max: pairwise fp16 DVE max over the jc blocks
    of eT -> emax[j',i]; per-i-tile PE transpose + one reduce_max give
    emax_col = e^m.  bvec numerators are then emax_col * exp(sc0-2.5)
    (multiplicative sc0 fold; sc0 columns from 16 tiny N=1 matmuls).
  - w_cq/w_q folded into chatT on the PSUM evacuation of the c transposes
    (tensor_scalar), so the score contraction emits the q@w_q row term.
  - c*c2q fused as (po*linv)*c via scalar_tensor_tensor straight from the
    c2q PSUM; 2 of 4 tiles per half on gpsimd from the normalized stage
    (gpsimd cannot read PSUM).  c*q2c in fp16, cast f16->f32 in the
    output DMA (SWDGE).
  - software pipelining: the next batch's q/c transposes + sc0 matmuls are
    emitted in 7 chunks at fixed points inside the current batch's body so
    the PE never drains (and the HAM clock stays unthrottled -- PE
    transposes do not count as busy for the clock gate).
  - partition-major DRAM layout ((p t) d) so each partition's DMA is one
    contiguous 8 KB descriptor instead of 8 strided 1 KB ones.
  - PSUM: 4 banks for the half-scores accumulator, 4 banks as a single
    rotating bufs=4 pool for everything else (transposes, sc0, c2q po
    tiles, q2c) -- per-tag slot allocation makes one shared tag mandatory.
  - DMA split: c-in/c-out on sync(SP), stage-out on scalar(ACT), q-in and
    c_h-in (f32->fp16 cast) and c4-out on gpsimd(SWDGE).

Measured: 118.0 us HW exec (vs 136.4 us v1 baseline), rel err 1.1e-04.
"""

import sys

if "/opt/trn_rl_repo" not in sys.path:
    sys.path.insert(0, "/opt/trn_rl_repo")

import numpy as np

import concourse.bass as bass
import concourse.tile as tile
from concourse import bacc, mybir
from concourse.bass import ds, ts
from concourse.masks import make_identity

B, CL, QL, D = 32, 1024, 512, 256
NCORES = 8
BS = B // NCORES  # batches per core
P = 128
F32 = mybir.dt.float32
F16 = mybir.dt.float16

NT = CL // P  # 8 i-tiles
NJ = QL // P  # 4 j-chunks
ND = D // P   # 2 d-chunks
NH = 2        # i-halves for the [j,i]-layout score matmul
IH = CL // NH  # 512
KPH = NT // NH  # i-tiles per half

Exp = mybir.ActivationFunctionType.Exp
AxX = mybir.AxisListType.X
Mult = mybir.AluOpType.mult
Add = mybir.AluOpType.add
Max = mybir.AluOpType.max


def build_bass(bs: int = BS):
    nc = bacc.Bacc(None)
    c_d = nc.declare_dram_parameter("c", [bs, CL, D], F32, isOutput=False)
    q_d = nc.declare_dram_parameter("q", [bs, QL, D], F32, isOutput=False)
    wc_d = nc.declare_dram_parameter("wc_cols", [P, ND], F32, isOutput=False)
    wq_d = nc.declare_dram_parameter("wq_cols", [P, ND], F32, isOutput=False)
    wcq_d = nc.declare_dram_parameter("wcq_cols", [P, ND], F32, isOutput=False)
    out_d = nc.declare_dram_parameter("out", [bs, CL, 4 * D], F32, isOutput=True)

    with tile.TileContext(nc) as tc:
        with (
            tc.tile_pool(name="consts", bufs=1) as consts,
            tc.tile_pool(name="io", bufs=2) as io,
            tc.tile_pool(name="ins", bufs=3) as ins,
            tc.tile_pool(name="work", bufs=2) as work,
            tc.tile_pool(name="ps_tT", bufs=1, space="PSUM") as ps_tT,
            tc.tile_pool(name="ps_w", bufs=4, space="PSUM") as ps_w,
        ):
            ident_h = consts.tile([P, P], F16)
            ones_f = consts.tile([P, P], F32)
            ones_h = consts.tile([P, P], F16)
            wc_sb = consts.tile([P, ND], F32)
            wq_sb = consts.tile([P, ND], F32)
            wcq_sb = consts.tile([P, ND], F32)
            wc_h = consts.tile([P, ND], F16)
            neg_shift = consts.tile([P, 1], F32)

            def emit_inputs(b):
                # c loaded once in f32; block0 of out is c verbatim
                c_sb = ins.tile([P, NT, D], F32, tag="c_sb")
                nc.sync.dma_start(
                    out=c_sb, in_=c_d[b].rearrange("(p t) d -> p t d", p=P)
                )
                ov = out_d[b].rearrange("(p t) x -> p t x", p=P)
                q_sb = ins.tile([P, NJ, D + 1], F16, tag="q_sb")
                c_h = ins.tile([P, NT, D], F16, tag="c_h")
                # q/c_h cast f32 -> fp16 in-flight (SWDGE)
                nc.gpsimd.dma_start(
                    out=q_sb[:, :, 0:D],
                    in_=q_d[b].rearrange("(p t) d -> p t d", p=P),
                )
                nc.gpsimd.dma_start(
                    out=c_h, in_=c_d[b].rearrange("(p t) d -> p t d", p=P)
                )
                nc.vector.memset(q_sb[:, :, D : D + 1], 1.0)
                nc.sync.dma_start(out=ov[:, :, 0:D], in_=c_sb)
                return c_sb, q_sb, c_h, ov

            def head_alloc():
                qT = work.tile([P, ND, QL], F16, tag="qT")
                cT = work.tile([P, ND, CL], F16, tag="cT")
                chatT = work.tile([P, ND, CL], F16, tag="chatT")
                return {"qT": qT, "cT": cT, "chatT": chatT}

            def head_qT(st, q_sb, dc):
                qps = ps_w.tile([P, QL], F16, tag="w")
                for jc in range(NJ):
                    nc.tensor.transpose(
                        qps[:, ts(jc, P)], q_sb[:, jc, ts(dc, P)], ident_h
                    )
                if dc == 0:
                    nc.scalar.copy(st["qT"][:, dc], qps)
                else:
                    nc.vector.tensor_copy(st["qT"][:, dc], qps)

            def head_cT(st, c_h, dc, h):
                pst = ps_w.tile([P, IH], F16, tag="w")
                for k in range(KPH):
                    it = h * KPH + k
                    nc.tensor.transpose(
                        pst[:, ts(k, P)], c_h[:, it, ts(dc, P)], ident_h
                    )
                sl = ds(h * IH, IH)
                nc.vector.tensor_scalar(
                    out=st["chatT"][:, dc, sl],
                    in0=pst,
                    scalar1=wcq_sb[:, dc : dc + 1],
                    scalar2=wq_sb[:, dc : dc + 1],
                    op0=Mult,
                    op1=Add,
                )
                nc.scalar.copy(st["cT"][:, dc, sl], pst)

            def head_sc0(st):
                sc0ps = ps_w.tile([P, NT], F32, tag="w")
                for it in range(NT):
                    for dc in range(ND):
                        nc.tensor.matmul(
                            sc0ps[:, it : it + 1],
                            st["cT"][:, dc, ts(it, P)],
                            wc_h[:, dc : dc + 1],
                            start=(dc == 0),
                            stop=(dc == ND - 1),
                        )
                esc = work.tile([P, NT], F32, tag="esc")
                nc.scalar.activation(esc, sc0ps, Exp, bias=neg_shift[:, 0:1])
                st["esc"] = esc

            def emit_head_full(q_sb, c_h):
                st = head_alloc()
                for dc in range(ND):
                    head_qT(st, q_sb, dc)
                for dc in range(ND):
                    for h in range(NH):
                        head_cT(st, c_h, dc, h)
                head_sc0(st)
                return st

            nc.scalar.dma_start(out=wc_sb, in_=wc_d[:])
            nc.scalar.dma_start(out=wq_sb, in_=wq_d[:])
            nc.scalar.dma_start(out=wcq_sb, in_=wcq_d[:])
            make_identity(nc, ident_h)
            nc.vector.memset(ones_f, 1.0)
            nc.vector.memset(ones_h, 1.0)
            nc.vector.memset(neg_shift, -2.5)
            nc.vector.tensor_copy(wc_h, wc_sb)

            pending = [emit_inputs(0)]

            for b in range(bs):
                c_sb, q_sb, c_h, ov = pending.pop(0)

                # prefetch up to two batches ahead
                if b == 0:
                    for nb in (1, 2):
                        if nb < bs:
                            pending.append(emit_inputs(nb))
                elif b + 2 < bs:
                    pending.append(emit_inputs(b + 2))

                if b == 0:
                    cur = emit_head_full(q_sb, c_h)
                if b + 1 < bs:
                    nxt = head_alloc()
                    nq, nch = pending[0][1], pending[0][2]
                    chunks = [
                        lambda: head_qT(nxt, nq, 0),
                        lambda: head_qT(nxt, nq, 1),
                        lambda: head_cT(nxt, nch, 0, 0),
                        lambda: head_cT(nxt, nch, 0, 1),
                        lambda: head_cT(nxt, nch, 1, 0),
                        lambda: head_cT(nxt, nch, 1, 1),
                        lambda: head_sc0(nxt),
                    ]
                else:
                    nxt, chunks = None, []

                def pump():
                    if chunks:
                        chunks.pop(0)()

                qT, cT, chatT, esc = (
                    cur["qT"], cur["cT"], cur["chatT"], cur["esc"]
                )
                emax = work.tile([P, CL], F16, tag="emax")
                emax_col = work.tile([P, NT], F32, tag="emaxc")
                ebv_h = work.tile([P, NT], F16, tag="ebvh")
                stage = io.tile([P, NT, 2 * D], F32, tag="stage")
                eTs = []

                for h in range(NH):
                    isl = ds(h * KPH, KPH)
                    csl = ds(h * IH, IH)
                    # ---- scores tT[j, i] for this half, exp per jc ----
                    tTq = ps_tT.tile([P, NJ, IH], F32, tag="tTq")
                    eT = work.tile([P, NJ, IH], F16, tag=f"eT{h}")
                    eTs.append(eT)
                    for jc in range(NJ):
                        for dc in range(ND):
                            nc.tensor.matmul(
                                tTq[:, jc],
                                qT[:, dc, ts(jc, P)],
                                chatT[:, dc, csl],
                                start=(dc == 0),
                                stop=(dc == ND - 1),
                            )
                        if jc % 2 == 1:
                            nc.scalar.activation(
                                eT[:, jc - 1 : jc + 1],
                                tTq[:, jc - 1 : jc + 1],
                                Exp,
                            )
                    pump()
                    # row max of e over the 4 jc blocks (fp16 SBUF; max of
                    # exp == exp of max, so emax = e^m directly)
                    tm0 = work.tile([P, IH], F16, tag="tm0")
                    nc.vector.tensor_tensor(tm0, eT[:, 0], eT[:, 1], op=Max)
                    tm1 = work.tile([P, IH], F16, tag="tm1")
                    nc.vector.tensor_tensor(tm1, eT[:, 2], eT[:, 3], op=Max)
                    nc.vector.tensor_tensor(emax[:, csl], tm0, tm1, op=Max)

                    # ---- e^m columns for this half: per-tile transpose of
                    # emax, then one reduce_max over j' ----
                    mps = ps_w.tile([P, KPH, P], F16, tag="w")
                    for k in range(KPH):
                        it = h * KPH + k
                        nc.tensor.transpose(
                            mps[:, k], emax[:, ts(it, P)], ident_h
                        )
                    nc.vector.reduce_max(emax_col[:, isl], mps, AxX)
                    nc.vector.tensor_mul(
                        ebv_h[:, isl], emax_col[:, isl], esc[:, isl]
                    )
                    pump()

                    # ---- c2q for this half ----
                    for k in range(KPH):
                        it = h * KPH + k
                        po = ps_w.tile([P, D + 1], F32, tag="w")
                        for jc in range(NJ):
                            nc.tensor.matmul(
                                po,
                                eT[:, jc, ts(k, P)],
                                q_sb[:, jc],
                                start=(jc == 0),
                                stop=(jc == NJ - 1),
                            )
                        linv = work.tile([P, 1], F32, tag="linv")
                        nc.vector.reciprocal(linv, po[:, D : D + 1])
                        if k < 3:
                            nc.scalar.mul(stage[:, it, 0:D], po[:, 0:D], linv)
                        else:
                            nc.vector.tensor_scalar_mul(
                                stage[:, it, 0:D], po[:, 0:D], linv
                            )
                        if k < 2:
                            nc.vector.scalar_tensor_tensor(
                                out=stage[:, it, D : 2 * D],
                                in0=po[:, 0:D],
                                scalar=linv,
                                in1=c_sb[:, it],
                                op0=Mult,
                                op1=Mult,
                            )
                        else:
                            # gpsimd cannot read PSUM; use normalized stage1
                            nc.gpsimd.tensor_mul(
                                stage[:, it, D : 2 * D],
                                stage[:, it, 0:D],
                                c_sb[:, it],
                            )
                        if k == 1:
                            pump()
                    if h == 0:
                        nc.scalar.dma_start(
                            out=ov[:, isl, D : 3 * D], in_=stage[:, isl]
                        )
                        pump()
                    else:
                        half = KPH // 2
                        nc.scalar.dma_start(
                            out=ov[:, ds(h * KPH, half), D : 3 * D],
                            in_=stage[:, ds(h * KPH, half)],
                        )
                        nc.scalar.dma_start(
                            out=ov[:, ds(h * KPH + half, half), D : 3 * D],
                            in_=stage[:, ds(h * KPH + half, half)],
                        )

                while chunks:
                    pump()
                cur = nxt

                # ---- q2c chain ----
                colsum = work.tile([P, 1], F32, tag="colsum")
                nc.vector.reduce_sum(colsum, ebv_h, AxX)
                ps_tot = ps_w.tile([P, 1], F32, tag="w")
                nc.tensor.matmul(ps_tot, ones_f, colsum, start=True, stop=True)
                totinv = work.tile([P, 1], F32, tag="totinv")
                nc.vector.reciprocal(totinv, ps_tot)
                ps_q2c = ps_w.tile([1, D], F32, tag="w")
                for it in range(NT):
                    nc.tensor.matmul(
                        ps_q2c,
                        ebv_h[:, it : it + 1],
                        c_h[:, it],
                        start=(it == 0),
                        stop=(it == NT - 1),
                    )
                q2c_row = work.tile([1, D], F32, tag="q2cr")
                nc.vector.tensor_scalar_mul(q2c_row, ps_q2c, totinv[0:1, 0:1])
                ps_q2cb = ps_w.tile([P, D], F32, tag="w")
                nc.tensor.matmul(
                    ps_q2cb, ones_f[0:1, :], q2c_row, start=True, stop=True
                )
                q2c_sb = work.tile([P, D], F16, tag="q2csb")
                nc.scalar.copy(q2c_sb, ps_q2cb)
                c4st = io.tile([P, NT, D], F16, tag="c4st")
                for it in range(KPH):
                    nc.vector.tensor_mul(c4st[:, it], c_h[:, it], q2c_sb)
                nc.gpsimd.dma_start(
                    out=ov[:, 0:KPH, 3 * D : 4 * D], in_=c4st[:, 0:KPH]
                )
                for it in range(KPH, NT):
                    nc.gpsimd.tensor_mul(c4st[:, it], c_h[:, it], q2c_sb)
                nc.gpsimd.dma_start(
                    out=ov[:, KPH:NT, 3 * D : 4 * D], in_=c4st[:, KPH:NT]
                )

    nc.compile()
    return nc


_NC_CACHE = {}


def _get_nc(bs: int = BS):
    if bs not in _NC_CACHE:
        _NC_CACHE[bs] = build_bass(bs)
    return _NC_CACHE[bs]


def _param_maps(w_c, w_q, w_cq):
    wc_cols = np.ascontiguousarray(np.asarray(w_c, np.float32).reshape(ND, P).T)
    wq_cols = np.ascontiguousarray(np.asarray(w_q, np.float32).reshape(ND, P).T)
    wcq_cols = np.ascontiguousarray(
        np.asarray(w_cq, np.float32).reshape(ND, P).T
    )
    return wc_cols, wq_cols, wcq_cols


def _run(c, q, w_c, w_q, w_cq, trace=False, **trace_kwargs):
    from concourse.bass_utils import run_bass_kernel_spmd

    c = np.asarray(c, np.float32)
    q = np.asarray(q, np.float32)
    wc_cols, wq_cols, wcq_cols = _param_maps(w_c, w_q, w_cq)

    nc = _get_nc(BS)
    in_maps = []
    for k in range(NCORES):
        in_maps.append(
            {
                "c": np.ascontiguousarray(c[k * BS : (k + 1) * BS]),
                "q": np.ascontiguousarray(q[k * BS : (k + 1) * BS]),
                "wc_cols": wc_cols,
                "wq_cols": wq_cols,
                "wcq_cols": wcq_cols,
            }
        )
    res = None
    last_err = None
    for attempt in range(3):
        try:
            res = run_bass_kernel_spmd(
                nc,
                in_maps,
                core_ids=list(range(NCORES)),
                trace=trace,
                **trace_kwargs,
            )
            break
        except Exception as e:  # transient device wedges clear on retry
            last_err = e
            if "UNRECOVERABLE" not in str(e) and "UNAVAILABLE" not in str(e):
                raise
    if res is None:
        raise last_err
    out = np.concatenate([res.results[k]["out"] for k in range(NCORES)], axis=0)
    return out, res


def kernel(c, q, w_c, b_c, w_q, b_q, w_cq, b_cq):
    # b_c/b_q/b_cq provably cancel in both softmaxes; output doesn't use them.
    out, _ = _run(c, q, w_c, w_q, w_cq)
    return out
